# revision 62
# baseline (speedup 1.0000x reference)
"""BertSelfAttention (relative_key_query) Trainium2 kernel, 8-core SPMD.

Sharding: core c -> (batch b = c//2, head-group hg = c%2, 8 heads each).
All matmul inputs bf16, PSUM fp32.

Key trick: the relative-position bias terms are Toeplitz.  Per 128-row tile
we compute a "band" matmul U[p, c] = q[l0+p] . P[band + c] on the PE, then
realign it with a single diagonal SBUF->SBUF DMA whose access pattern steps
partition-pitch-minus-one elements per partition (a shear).  The q-side bias
is computed in (l, r) orientation, sheared, then transposed into scores via
PE transpose ops; the k-side bias shears directly into (r, l) orientation.
Scores are built transposed (scoresT[r, l]) so that probs@v needs no
transpose and the softmax denominator falls out of a ones-column in v.

Pipeline layout (this version): single-head software pipeline -- the band
matmuls + evicts + U shears of head h+1 are interleaved into the scores
loop of head h so the PE never drains.  Per rt block: the kq scores land
in two single-bank PSUM halves, the sheared U band is PE-transposed
(bf16) into one psU bank, the V band shears into SBUF ahead of time
(SWDGE, hoisted to head start), and DVE combines them in two adds
(psU+Vsh all-bf16, then psS+sc1 per half -- a TensorTensor may read only
one PSUM operand).  The attention mask rides the EXP activation bias.
Band-piece evicts rotate DVE/ACT/ACT (DVE also owns the adds) and are
emitted after the adds so the adds sit at the DVE queue head.  ctx runs
v-stationary (ctxT[d, l], one matmul per 512 l-cols) with emission
delayed two iterations so the in-order PE queue never waits on EXP; the
output ships unnormalized (64 dims + denominator row per head) and the
softmax division happens on host in assemble_output.

PSUM groups must be balanced (one start, one stop, consumers released at
stop) and a matmul may not cross a PSUM bank boundary; start_tensor_calc
zeroes bank-granularly (2KB).
"""

import numpy as np
import ml_dtypes
from contextlib import ExitStack

import concourse.bass as bass
import concourse.mybir as mybir
import concourse.tile as tile
from concourse.masks import make_identity

bf16 = ml_dtypes.bfloat16
F32 = mybir.dt.float32
BF16 = mybir.dt.bfloat16

B, S, H = 4, 1024, 1024
NH, HD = 16, 64
MAXPOS = 1024
NCORES = 8
HPC = 8          # heads per core
DHC = HPC * HD   # 512 out-dims per core
KA = H + 8       # augmented contraction (bias fold), 1032
NT = S // 128    # 8 tiles of 128 along sequence
BW = 1152        # band width
BPW = 384        # band piece width (3 pieces per band tile, 1 PSUM bank each)
VW = 520         # v block width: 8 heads x 65 (64 dims + ones col)
OW = HPC * 65    # out width per core: 8 heads x (64 dims + denom) = 520


def _emit(tc, io):
    nc = tc.nc
    ctx = ExitStack()
    with ctx:
        # ---------------- persistent tiles ----------------
        pers = ctx.enter_context(tc.tile_pool(name="pers", bufs=1))
        # P tables replicated into both 64-partition halves so that
        # band/qk matmuls can match any head's base partition.
        PT_sb = pers.tile([128, 2048], BF16)
        PrT_sb = pers.tile([128, 2048], BF16)
        ident = pers.tile([128, 128], BF16)
        mask_sb = pers.tile([128, 8], F32)
        qT_sb = pers.tile([128, 4 * 1024], BF16)   # dh-chunk c at cols 1024c
        kT_sb = pers.tile([128, 4 * 1024], BF16)
        v_sb = pers.tile([128, NT * VW], BF16)     # r-chunk rc at cols VW*rc

        nc.sync.dma_start(PT_sb[0:64, :], io["PT"][:])
        nc.sync.dma_start(PT_sb[64:128, :], io["PT"][:])
        nc.sync.dma_start(PrT_sb[0:64, :], io["PrT"][:])
        nc.sync.dma_start(PrT_sb[64:128, :], io["PrT"][:])
        nc.sync.dma_start(mask_sb[:], io["maskT"][:])
        make_identity(nc, ident[:])

        # ---------------- projections ----------------
        with tc.tile_pool(name="proj", bufs=1) as pj, \
             tc.tile_pool(name="projps", bufs=2, space="PSUM") as pjps:
            hs_m = pj.tile([128, 8 * 1024], BF16)   # k-chunk kc at cols 1024kc
            hs_t = pj.tile([8, 1024], BF16)
            wq_m = pj.tile([128, 8 * DHC], BF16)
            wk_m = pj.tile([128, 8 * DHC], BF16)
            wv_m = pj.tile([128, 8 * VW], BF16)
            wv_t = pj.tile([8, VW], BF16)
            bq_sb = pj.tile([128, 4], F32)
            bk_sb = pj.tile([128, 4], F32)

            hsd, wqd, wkd, wvd = io["hsT"], io["wqT"], io["wkT"], io["wvT"]
            # hsT dram [1032, 1024] -> chunked SBUF layout
            nc.sync.dma_start(
                hs_m[:],
                bass.AP(hsd.ap().tensor, 0, [[1024, 128], [128 * 1024, 8], [1, 1024]]))
            nc.sync.dma_start(hs_t[:], hsd.ap()[1024:1032, :])
            for wm, wd, wcols in ((wq_m, wqd, DHC), (wk_m, wkd, DHC),
                                  (wv_m, wvd, VW)):
                nc.sync.dma_start(
                    wm[:],
                    bass.AP(wd.ap().tensor, 0,
                            [[wcols, 128], [128 * wcols, 8], [1, wcols]]))
            nc.sync.dma_start(wv_t[:], wvd.ap()[1024:1032, :])
            nc.sync.dma_start(bq_sb[:], io["bq"].ap()[:])
            nc.sync.dma_start(bk_sb[:], io["bk"].ap()[:])

            # q/k: out[dh-chunk c partitions, tokens]; bias rides the evict
            # activation (per-partition = per-out-dim)
            for wm, bias, dst in ((wq_m, bq_sb, qT_sb), (wk_m, bk_sb, kT_sb)):
                for c in range(4):
                    for th in range(2):
                        ps = pjps.tile([128, 512], F32, tag="pps")
                        for kc in range(8):
                            nc.tensor.matmul(
                                ps[:],
                                wm[:, 512 * kc + 128 * c:512 * kc + 128 * c + 128],
                                hs_m[:, 1024 * kc + 512 * th:1024 * kc + 512 * th + 512],
                                start=(kc == 0), stop=(kc == 7))
                        nc.scalar.activation(
                            dst[:, 1024 * c + 512 * th:1024 * c + 512 * th + 512],
                            ps[:], mybir.ActivationFunctionType.Identity,
                            bias=bias[:, c:c + 1])
            # v: out[token-chunk rc partitions, VW]
            for rc in range(8):
                psa = pjps.tile([128, 512], F32, tag="pps")
                psb = pjps.tile([128, 8], F32, tag="ppsb")
                for kc in range(8):
                    lhs = hs_m[:, 1024 * kc + 128 * rc:1024 * kc + 128 * rc + 128]
                    nc.tensor.matmul(psa[:], lhs,
                                     wv_m[:, VW * kc:VW * kc + 512],
                                     start=(kc == 0), stop=False)
                    nc.tensor.matmul(psb[:], lhs,
                                     wv_m[:, VW * kc + 512:VW * kc + VW],
                                     start=(kc == 0), stop=False)
                nc.tensor.matmul(psa[:], hs_t[:, 128 * rc:128 * rc + 128],
                                 wv_t[:, 0:512], start=False, stop=True)
                nc.tensor.matmul(psb[:], hs_t[:, 128 * rc:128 * rc + 128],
                                 wv_t[:, 512:VW], start=False, stop=True)
                nc.vector.tensor_copy(v_sb[:, VW * rc:VW * rc + 512], psa[:])
                nc.vector.tensor_copy(v_sb[:, VW * rc + 512:VW * rc + VW], psb[:])

        # ---------------- per-head attention ----------------
        # PSUM budget (8 banks): band pieces f32[128,384] x3bufs = 3,
        # psS halves f32[128,512] x2 = 2, psU bf16[128,1024] x1 = 1,
        # ctx f32[65,512] x2 = 2.
        bands = ctx.enter_context(tc.tile_pool(name="bands", bufs=2))
        ubp = ctx.enter_context(tc.tile_pool(name="ubp", bufs=3))
        shrd = ctx.enter_context(tc.tile_pool(name="shrd", bufs=2))
        work = ctx.enter_context(tc.tile_pool(name="work", bufs=2))
        bandp = ctx.enter_context(tc.tile_pool(name="bandp", bufs=3, space="PSUM"))
        sps = ctx.enter_context(tc.tile_pool(name="sps", bufs=2, space="PSUM"))
        ups = ctx.enter_context(tc.tile_pool(name="ups", bufs=1, space="PSUM"))
        ctxps = ctx.enter_context(tc.tile_pool(name="ctxps", bufs=1, space="PSUM"))

        outd = io["out"]
        # GPSIMD cannot access PSUM (walrus verifier) -> evicts ride DVE/ACT.
        # DVE also carries the bias adds, so it only gets 1 piece in 3.
        evict_engines = (nc.vector, nc.scalar, nc.scalar)
        evict_n = [0]

        def head_tiles(h):
            Vb = bands.tile([128, NT * BW], BF16, tag="Vb")
            Ush = shrd.tile([128, NT * 1024], BF16, tag="Ush")
            Vsh = shrd.tile([128, NT * 1024], BF16, tag="Vsh")
            return (Vb, Ush, Vsh)

        def emit_band_side(h, t, tiles, side):
            """Band matmuls + evicts for one side (U or V) of tile t of head
            h; the U side is followed by its shear DMA.  The two sides are
            emitted with other PE work between them so the 3-buffer band
            PSUM pool never starves the PE."""
            Vb, Ush, Vsh = tiles
            hc, ho = h // 2, 64 * (h % 2)
            s0 = 896 - 128 * t
            if side == "U":
                src_sb, tbl, dst = qT_sb, PrT_sb, None
                Ubt = ubp.tile([128, BW], BF16, tag="Ubt")
            else:
                src_sb, tbl, dst = kT_sb, PT_sb, Vb
                Ubt = None
            lhsT = src_sb[ho:ho + 64, 1024 * hc + 128 * t:1024 * hc + 128 * t + 128]
            for k in range(3):
                ps = bandp.tile([128, BPW], F32, tag="bp")
                nc.tensor.matmul(ps[:], lhsT,
                                 tbl[ho:ho + 64, s0 + BPW * k:s0 + BPW * (k + 1)])
                eng = evict_engines[evict_n[0] % 3]
                evict_n[0] += 1
                if dst is None:
                    dpiece = Ubt[:, BPW * k:BPW * (k + 1)]
                else:
                    dpiece = dst[:, BW * t + BPW * k:BW * t + BPW * (k + 1)]
                if eng is nc.scalar:
                    eng.copy(dpiece, ps[:])
                else:
                    eng.tensor_copy(dpiece, ps[:])
            if dst is None:
                # U shear tile t: partition p reads cols (127 - p ..)
                sap = Ubt[:]
                diag = bass.AP(sap.tensor, sap.offset + 127,
                               [[BW - 1, 128], [1, 1024]])
                dap = Ush[:]
                dstap = bass.AP(dap.tensor, dap.offset + 1024 * t,
                                [[NT * 1024, 128], [1, 1024]])
                nc.sync.dma_start(dstap, diag)

        def emit_band_tile(h, t, tiles):
            emit_band_side(h, t, tiles, "U")
            emit_band_side(h, t, tiles, "V")

        def emit_ctx(h, rt, probs_t, cpsAB):
            # v-stationary orientation: ctxT[d, l] = sum_r v[r, d] probsT[r, l]
            # accumulated over rt blocks.  One matmul per 512 l-columns (one
            # PSUM bank each), balanced start/stop per tile.
            for hf in range(2):
                nc.tensor.matmul(
                    cpsAB[hf][:],
                    v_sb[:, VW * rt + 65 * h:VW * rt + 65 * h + 65],
                    probs_t[:, 512 * hf:512 * hf + 512],
                    start=(rt == 0), stop=(rt == NT - 1))

        def emit_head(h, tiles, next_tiles):
            """Scores/softmax/ctx for head h; bands for head h+1 interleaved."""
            Vb, Ush, Vsh = tiles
            hc, ho = h // 2, 64 * (h % 2)
            cpsA = ctxps.tile([65, 512], F32, tag="ctxA")
            cpsB = ctxps.tile([65, 512], F32, tag="ctxB")
            cpsAB = (cpsA, cpsB)
            # V bias shears: all inputs (Vb) were evicted during head h-1,
            # so hoist the SWDGE work ahead of the rt loop
            for rt in range(NT):
                sap = Vb[:]
                diag = bass.AP(sap.tensor, sap.offset + BW * rt + 127,
                               [[NT * BW - 1, 128], [1, 1024]])
                nc.gpsimd.dma_start(Vsh[:, 1024 * rt:1024 * rt + 1024], diag)
            pending = []  # (rt, probs_tile) awaiting ctx emission (depth 2)
            for rt in range(NT):
                # scoresT (k . q) per 512-half
                lhsT = kT_sb[ho:ho + 64, 1024 * hc + 128 * rt:1024 * hc + 128 * rt + 128]
                halves = []
                for hf in range(2):
                    psS = sps.tile([128, 512], F32, tag="S")
                    nc.tensor.matmul(
                        psS[:], lhsT,
                        qT_sb[ho:ho + 64, 1024 * hc + 512 * hf:1024 * hc + 512 * hf + 512])
                    halves.append(psS)
                # transposed U bias for this rt: bf16 PE transposes into one
                # psU bank (first carries start, bank zeroing covers all)
                psU = ups.tile([128, 1024], BF16, tag="U")
                for lt in range(NT):
                    nc.tensor.matmul(
                        psU[:, 128 * lt:128 * lt + 128],
                        Ush[:, 1024 * lt + 128 * rt:1024 * lt + 128 * rt + 128],
                        ident[:], is_transpose=True,
                        start=(lt == 0), stop=(lt == NT - 1))
                # sc1 = psU + Vsh (all-bf16 DVE add), sc = psS + sc1, exp.
                # Emitted BEFORE the band evicts so the adds sit at the DVE
                # queue head and release psS/psU promptly.
                sc1 = work.tile([128, 1024], BF16, tag="sc1")
                nc.vector.tensor_add(sc1[:], psU[:],
                                     Vsh[:, 1024 * rt:1024 * rt + 1024])
                sc = work.tile([128, 1024], BF16, tag="sc")
                for hf in range(2):
                    nc.vector.tensor_add(sc[:, 512 * hf:512 * hf + 512],
                                         halves[hf][:],
                                         sc1[:, 512 * hf:512 * hf + 512])
                probs_t = work.tile([128, 1024], BF16, tag="probs", bufs=4)
                nc.scalar.activation(probs_t[:], sc[:],
                                     mybir.ActivationFunctionType.Exp,
                                     bias=mask_sb[:, rt:rt + 1], scale=0.125)
                pending.append((rt, probs_t))
                # U band triple, then ctx, then V band triple: the ctx block
                # between the triples lets the evicts drain the band pool
                if next_tiles is not None:
                    emit_band_side(h + 1, rt, next_tiles, "U")
                if len(pending) > 2:
                    prt, pt = pending.pop(0)
                    emit_ctx(h, prt, pt, cpsAB)
                if next_tiles is not None:
                    emit_band_side(h + 1, rt, next_tiles, "V")
            for prt, pt in pending:
                emit_ctx(h, prt, pt, cpsAB)
            # ship unnormalized ctxT (64 dims + denom row per head) to DRAM
            outsb = work.tile([65, 1024], F32, tag="outsb")
            nc.scalar.copy(outsb[:, 0:512], cpsAB[0][:])
            nc.scalar.copy(outsb[:, 512:1024], cpsAB[1][:])
            nc.sync.dma_start(outd.ap()[65 * h:65 * h + 65, :], outsb[:])

        tiles = head_tiles(0)
        for t in range(NT):
            emit_band_tile(0, t, tiles)
        for h in range(HPC):
            next_tiles = head_tiles(h + 1) if h + 1 < HPC else None
            emit_head(h, tiles, next_tiles)
            tiles = next_tiles


def build_module():
    from concourse import bacc
    nc = bacc.Bacc("TRN2", target_bir_lowering=False)
    io = {
        "hsT": nc.dram_tensor("hsT", [KA, S], BF16, kind="ExternalInput"),
        "wqT": nc.dram_tensor("wqT", [KA, DHC], BF16, kind="ExternalInput"),
        "wkT": nc.dram_tensor("wkT", [KA, DHC], BF16, kind="ExternalInput"),
        "wvT": nc.dram_tensor("wvT", [KA, VW], BF16, kind="ExternalInput"),
        "bq": nc.dram_tensor("bq", [128, 4], F32, kind="ExternalInput"),
        "bk": nc.dram_tensor("bk", [128, 4], F32, kind="ExternalInput"),
        "PT": nc.dram_tensor("PT", [64, 2048], BF16, kind="ExternalInput"),
        "PrT": nc.dram_tensor("PrT", [64, 2048], BF16, kind="ExternalInput"),
        "maskT": nc.dram_tensor("maskT", [128, 8], F32, kind="ExternalInput"),
        "out": nc.dram_tensor("out", [OW, S], F32, kind="ExternalOutput"),
    }
    with tile.TileContext(nc) as tc:
        _emit(tc, io)
    nc.compile()
    return nc


def shard_inputs(hidden_states, attention_mask, wq, bq, wk, bk, wv, bv, dist_emb):
    """Full fp32 inputs -> per-core in_maps (bf16 where appropriate)."""
    hidden_states = np.asarray(hidden_states, np.float32)
    attention_mask = np.asarray(attention_mask, np.float32)
    wq, bq = np.asarray(wq, np.float32), np.asarray(bq, np.float32)
    wk, bk = np.asarray(wk, np.float32), np.asarray(bk, np.float32)
    wv, bv = np.asarray(wv, np.float32), np.asarray(bv, np.float32)
    dist_emb = np.asarray(dist_emb, np.float32)

    PT = np.zeros((64, 2048), bf16)
    PT[:, :2047] = dist_emb.T.astype(bf16)
    PrT = np.zeros((64, 2048), bf16)
    PrT[:, :2047] = dist_emb[::-1].T.astype(bf16)

    in_maps = []
    for c in range(NCORES):
        b, hg = c // 2, c % 2
        sl = slice(DHC * hg, DHC * (hg + 1))

        hsT = np.zeros((KA, S), bf16)
        hsT[:H] = hidden_states[b].T.astype(bf16)
        hsT[H] = bf16(1.0)

        wqT = np.zeros((KA, DHC), bf16)
        wqT[:H] = wq[sl].T.astype(bf16)
        wkT = np.zeros((KA, DHC), bf16)
        wkT[:H] = wk[sl].T.astype(bf16)
        bq_t = np.ascontiguousarray(bq[sl].reshape(4, 128).T).astype(np.float32)
        bk_t = np.ascontiguousarray(bk[sl].reshape(4, 128).T).astype(np.float32)

        wvT = np.zeros((KA, VW), bf16)
        for h in range(HPC):
            cs = 65 * h
            wvT[:H, cs:cs + 64] = wv[DHC * hg + 64 * h:DHC * hg + 64 * h + 64].T.astype(bf16)
            wvT[H, cs:cs + 64] = bv[DHC * hg + 64 * h:DHC * hg + 64 * h + 64].astype(bf16)
            wvT[H, cs + 64] = bf16(1.0)

        # mask rides the EXP activation bias: exp(0.125*sc + mask)
        maskT = np.ascontiguousarray(
            attention_mask[b, 0, 0].reshape(8, 128).T).astype(np.float32)

        in_maps.append({"hsT": hsT, "wqT": wqT, "wkT": wkT, "wvT": wvT,
                        "bq": bq_t, "bk": bk_t,
                        "PT": PT.copy(), "PrT": PrT.copy(), "maskT": maskT})
    return in_maps


def assemble_output(results):
    out = np.zeros((B, S, H), np.float32)
    for c in range(NCORES):
        b, hg = c // 2, c % 2
        r = results[c]["out"]  # [8*65, S] unnormalized ctxT + denom rows
        for h in range(HPC):
            den = r[65 * h + 64]
            out[b, :, DHC * hg + 64 * h:DHC * hg + 64 * h + 64] = \
                (r[65 * h:65 * h + 64] / den).T
    return out


_NC_CACHE = {}


def kernel(**inputs):
    from concourse import bass_utils
    if "nc" not in _NC_CACHE:
        _NC_CACHE["nc"] = build_module()
    nc = _NC_CACHE["nc"]
    in_maps = shard_inputs(**inputs)
    res = bass_utils.run_bass_kernel_spmd(nc, in_maps, core_ids=list(range(NCORES)))
    return assemble_output(res.results)


# revision 63
# speedup vs baseline: 1.0107x; 1.0107x over previous
"""BertSelfAttention (relative_key_query) Trainium2 kernel, 8-core SPMD.

Sharding: core c -> (batch b = c//2, head-group hg = c%2, 8 heads each).
All matmul inputs bf16, PSUM fp32.

Key trick: the relative-position bias terms are Toeplitz.  Per 128-row tile
we compute a "band" matmul U[p, c] = q[l0+p] . P[band + c] on the PE, then
realign it with a single diagonal SBUF->SBUF DMA whose access pattern steps
partition-pitch-minus-one elements per partition (a shear).  The q-side bias
is computed in (l, r) orientation, sheared, then transposed into scores via
PE transpose ops; the k-side bias shears directly into (r, l) orientation.
Scores are built transposed (scoresT[r, l]) so that probs@v needs no
transpose and the softmax denominator falls out of a ones-column in v.

Pipeline layout (this version): single-head software pipeline -- the band
matmuls + evicts + U shears of head h+1 are interleaved into the scores
loop of head h so the PE never drains.  Per rt block: the kq scores land
in two single-bank PSUM halves, the sheared U band is PE-transposed
(bf16) into one psU bank, the V band shears into SBUF ahead of time
(SWDGE, hoisted to head start), and DVE combines them in two adds
(psU+Vsh all-bf16, then psS+sc1 per half -- a TensorTensor may read only
one PSUM operand).  The attention mask rides the EXP activation bias.
Band-piece evicts rotate DVE/ACT/ACT (DVE also owns the adds) and are
emitted after the adds so the adds sit at the DVE queue head.  ctx runs
v-stationary (ctxT[d, l], one matmul per 512 l-cols) with emission
delayed two iterations so the in-order PE queue never waits on EXP; the
output ships unnormalized (64 dims + denominator row per head) and the
softmax division happens on host in assemble_output.

PSUM groups must be balanced (one start, one stop, consumers released at
stop) and a matmul may not cross a PSUM bank boundary; start_tensor_calc
zeroes bank-granularly (2KB).
"""

import numpy as np
import ml_dtypes
from contextlib import ExitStack

import concourse.bass as bass
import concourse.mybir as mybir
import concourse.tile as tile
from concourse.masks import make_identity

bf16 = ml_dtypes.bfloat16
F32 = mybir.dt.float32
BF16 = mybir.dt.bfloat16

B, S, H = 4, 1024, 1024
NH, HD = 16, 64
MAXPOS = 1024
NCORES = 8
HPC = 8          # heads per core
DHC = HPC * HD   # 512 out-dims per core
KA = H + 8       # augmented contraction (bias fold), 1032
NT = S // 128    # 8 tiles of 128 along sequence
BW = 1152        # band width
BPW = 384        # band piece width (3 pieces per band tile, 1 PSUM bank each)
VW = 520         # v block width: 8 heads x 65 (64 dims + ones col)
OW = HPC * 65    # out width per core: 8 heads x (64 dims + denom) = 520


def _emit(tc, io):
    nc = tc.nc
    ctx = ExitStack()
    with ctx:
        # ---------------- persistent tiles ----------------
        pers = ctx.enter_context(tc.tile_pool(name="pers", bufs=1))
        # P tables replicated into both 64-partition halves so that
        # band/qk matmuls can match any head's base partition.
        PT_sb = pers.tile([128, 2048], BF16)
        PrT_sb = pers.tile([128, 2048], BF16)
        ident = pers.tile([128, 128], BF16)
        mask_sb = pers.tile([128, 8], F32)
        qT_sb = pers.tile([128, 4 * 1024], BF16)   # dh-chunk c at cols 1024c
        kT_sb = pers.tile([128, 4 * 1024], BF16)
        v_sb = pers.tile([128, NT * VW], BF16)     # r-chunk rc at cols VW*rc

        nc.sync.dma_start(PT_sb[0:64, :], io["PT"][:])
        nc.sync.dma_start(PT_sb[64:128, :], io["PT"][:])
        nc.sync.dma_start(PrT_sb[0:64, :], io["PrT"][:])
        nc.sync.dma_start(PrT_sb[64:128, :], io["PrT"][:])
        nc.sync.dma_start(mask_sb[:], io["maskT"][:])
        make_identity(nc, ident[:])

        # ---------------- projections ----------------
        with tc.tile_pool(name="proj", bufs=1) as pj, \
             tc.tile_pool(name="projps", bufs=2, space="PSUM") as pjps:
            hs_m = pj.tile([128, 8 * 1024], BF16)   # k-chunk kc at cols 1024kc
            hs_t = pj.tile([8, 1024], BF16)
            wq_m = pj.tile([128, 8 * DHC], BF16)
            wk_m = pj.tile([128, 8 * DHC], BF16)
            wv_m = pj.tile([128, 8 * VW], BF16)
            wv_t = pj.tile([8, VW], BF16)
            bq_sb = pj.tile([128, 4], F32)
            bk_sb = pj.tile([128, 4], F32)

            hsd, wqd, wkd, wvd = io["hsT"], io["wqT"], io["wkT"], io["wvT"]
            # hsT dram [1032, 1024] -> chunked SBUF layout
            nc.sync.dma_start(
                hs_m[:],
                bass.AP(hsd.ap().tensor, 0, [[1024, 128], [128 * 1024, 8], [1, 1024]]))
            nc.sync.dma_start(hs_t[:], hsd.ap()[1024:1032, :])
            for wm, wd, wcols in ((wq_m, wqd, DHC), (wk_m, wkd, DHC),
                                  (wv_m, wvd, VW)):
                nc.sync.dma_start(
                    wm[:],
                    bass.AP(wd.ap().tensor, 0,
                            [[wcols, 128], [128 * wcols, 8], [1, wcols]]))
            nc.sync.dma_start(wv_t[:], wvd.ap()[1024:1032, :])
            nc.sync.dma_start(bq_sb[:], io["bq"].ap()[:])
            nc.sync.dma_start(bk_sb[:], io["bk"].ap()[:])

            # q/k: out[dh-chunk c partitions, tokens]; bias rides the evict
            # activation (per-partition = per-out-dim)
            for wm, bias, dst in ((wq_m, bq_sb, qT_sb), (wk_m, bk_sb, kT_sb)):
                for c in range(4):
                    for th in range(2):
                        ps = pjps.tile([128, 512], F32, tag="pps")
                        for kc in range(8):
                            nc.tensor.matmul(
                                ps[:],
                                wm[:, 512 * kc + 128 * c:512 * kc + 128 * c + 128],
                                hs_m[:, 1024 * kc + 512 * th:1024 * kc + 512 * th + 512],
                                start=(kc == 0), stop=(kc == 7))
                        nc.scalar.activation(
                            dst[:, 1024 * c + 512 * th:1024 * c + 512 * th + 512],
                            ps[:], mybir.ActivationFunctionType.Identity,
                            bias=bias[:, c:c + 1])
            # v: out[token-chunk rc partitions, VW]
            for rc in range(8):
                psa = pjps.tile([128, 512], F32, tag="pps")
                psb = pjps.tile([128, 8], F32, tag="ppsb")
                for kc in range(8):
                    lhs = hs_m[:, 1024 * kc + 128 * rc:1024 * kc + 128 * rc + 128]
                    nc.tensor.matmul(psa[:], lhs,
                                     wv_m[:, VW * kc:VW * kc + 512],
                                     start=(kc == 0), stop=False)
                    nc.tensor.matmul(psb[:], lhs,
                                     wv_m[:, VW * kc + 512:VW * kc + VW],
                                     start=(kc == 0), stop=False)
                nc.tensor.matmul(psa[:], hs_t[:, 128 * rc:128 * rc + 128],
                                 wv_t[:, 0:512], start=False, stop=True)
                nc.tensor.matmul(psb[:], hs_t[:, 128 * rc:128 * rc + 128],
                                 wv_t[:, 512:VW], start=False, stop=True)
                nc.vector.tensor_copy(v_sb[:, VW * rc:VW * rc + 512], psa[:])
                nc.vector.tensor_copy(v_sb[:, VW * rc + 512:VW * rc + VW], psb[:])

        # ---------------- per-head attention ----------------
        # PSUM budget (8 banks): band pieces f32[128,384] x3bufs = 3,
        # psS halves f32[128,512] x2 = 2, psU bf16[128,1024] x1 = 1,
        # ctx f32[65,512] x2 = 2.
        bands = ctx.enter_context(tc.tile_pool(name="bands", bufs=2))
        ubp = ctx.enter_context(tc.tile_pool(name="ubp", bufs=3))
        shrd = ctx.enter_context(tc.tile_pool(name="shrd", bufs=2))
        work = ctx.enter_context(tc.tile_pool(name="work", bufs=2))
        bandp = ctx.enter_context(tc.tile_pool(name="bandp", bufs=3, space="PSUM"))
        sps = ctx.enter_context(tc.tile_pool(name="sps", bufs=2, space="PSUM"))
        ups = ctx.enter_context(tc.tile_pool(name="ups", bufs=1, space="PSUM"))
        ctxps = ctx.enter_context(tc.tile_pool(name="ctxps", bufs=1, space="PSUM"))

        outd = io["out"]
        # GPSIMD cannot access PSUM (walrus verifier) -> evicts ride DVE/ACT.
        # DVE also carries the bias adds, so it only gets 1 piece in 3.
        evict_engines = (nc.vector, nc.scalar, nc.scalar)
        evict_n = [0]

        def head_tiles(h):
            Vb = bands.tile([128, NT * BW], BF16, tag="Vb")
            Ush = shrd.tile([128, NT * 1024], BF16, tag="Ush")
            Vsh = shrd.tile([128, NT * 1024], BF16, tag="Vsh")
            return (Vb, Ush, Vsh)

        def emit_band_tile(h, t, tiles):
            """Band matmuls + evicts for tile t of head h, then the U shear."""
            Vb, Ush, Vsh = tiles
            hc, ho = h // 2, 64 * (h % 2)
            s0 = 896 - 128 * t
            Ubt = ubp.tile([128, BW], BF16, tag="Ubt")
            for src_sb, tbl, dst in ((qT_sb, PrT_sb, None), (kT_sb, PT_sb, Vb)):
                lhsT = src_sb[ho:ho + 64, 1024 * hc + 128 * t:1024 * hc + 128 * t + 128]
                for k in range(3):
                    ps = bandp.tile([128, BPW], F32, tag="bp")
                    nc.tensor.matmul(ps[:], lhsT,
                                     tbl[ho:ho + 64, s0 + BPW * k:s0 + BPW * (k + 1)])
                    eng = evict_engines[evict_n[0] % 3]
                    evict_n[0] += 1
                    if dst is None:
                        dpiece = Ubt[:, BPW * k:BPW * (k + 1)]
                    else:
                        dpiece = dst[:, BW * t + BPW * k:BW * t + BPW * (k + 1)]
                    if eng is nc.scalar:
                        eng.copy(dpiece, ps[:])
                    else:
                        eng.tensor_copy(dpiece, ps[:])
                if dst is None:
                    # U shear tile t: partition p reads cols (127 - p ..)
                    sap = Ubt[:]
                    diag = bass.AP(sap.tensor, sap.offset + 127,
                                   [[BW - 1, 128], [1, 1024]])
                    dap = Ush[:]
                    dstap = bass.AP(dap.tensor, dap.offset + 1024 * t,
                                    [[NT * 1024, 128], [1, 1024]])
                    nc.sync.dma_start(dstap, diag)

        def emit_ctx(h, rt, probs_t, cpsAB):
            # v-stationary orientation: ctxT[d, l] = sum_r v[r, d] probsT[r, l]
            # accumulated over rt blocks.  One matmul per 512 l-columns (one
            # PSUM bank each), balanced start/stop per tile.
            for hf in range(2):
                nc.tensor.matmul(
                    cpsAB[hf][:],
                    v_sb[:, VW * rt + 65 * h:VW * rt + 65 * h + 65],
                    probs_t[:, 512 * hf:512 * hf + 512],
                    start=(rt == 0), stop=(rt == NT - 1))

        def emit_head(h, tiles, next_tiles):
            """Scores/softmax/ctx for head h; bands for head h+1 interleaved."""
            Vb, Ush, Vsh = tiles
            hc, ho = h // 2, 64 * (h % 2)
            cpsA = ctxps.tile([65, 512], F32, tag="ctxA")
            cpsB = ctxps.tile([65, 512], F32, tag="ctxB")
            cpsAB = (cpsA, cpsB)
            # V bias shears: all inputs (Vb) were evicted during head h-1,
            # so hoist the SWDGE work ahead of the rt loop
            for rt in range(NT):
                sap = Vb[:]
                diag = bass.AP(sap.tensor, sap.offset + BW * rt + 127,
                               [[NT * BW - 1, 128], [1, 1024]])
                nc.gpsimd.dma_start(Vsh[:, 1024 * rt:1024 * rt + 1024], diag)
            pending = []  # (rt, probs_tile) awaiting ctx emission (depth 2)
            for rt in range(NT):
                # scoresT (k . q) per 512-half
                lhsT = kT_sb[ho:ho + 64, 1024 * hc + 128 * rt:1024 * hc + 128 * rt + 128]
                halves = []
                for hf in range(2):
                    psS = sps.tile([128, 512], F32, tag="S")
                    nc.tensor.matmul(
                        psS[:], lhsT,
                        qT_sb[ho:ho + 64, 1024 * hc + 512 * hf:1024 * hc + 512 * hf + 512])
                    halves.append(psS)
                # transposed U bias for this rt: bf16 PE transposes into one
                # psU bank (first carries start, bank zeroing covers all)
                psU = ups.tile([128, 1024], BF16, tag="U")
                for lt in range(NT):
                    nc.tensor.matmul(
                        psU[:, 128 * lt:128 * lt + 128],
                        Ush[:, 1024 * lt + 128 * rt:1024 * lt + 128 * rt + 128],
                        ident[:], is_transpose=True,
                        start=(lt == 0), stop=(lt == NT - 1))
                # sc1 = psU + Vsh (all-bf16 DVE add), sc = psS + sc1, exp.
                # Emitted BEFORE the band evicts so the adds sit at the DVE
                # queue head and release psS/psU promptly.
                sc1 = work.tile([128, 1024], BF16, tag="sc1")
                nc.vector.tensor_add(sc1[:], psU[:],
                                     Vsh[:, 1024 * rt:1024 * rt + 1024])
                sc = work.tile([128, 1024], BF16, tag="sc")
                for hf in range(2):
                    nc.vector.tensor_add(sc[:, 512 * hf:512 * hf + 512],
                                         halves[hf][:],
                                         sc1[:, 512 * hf:512 * hf + 512])
                probs_t = work.tile([128, 1024], BF16, tag="probs", bufs=4)
                nc.scalar.activation(probs_t[:], sc[:],
                                     mybir.ActivationFunctionType.Exp,
                                     bias=mask_sb[:, rt:rt + 1], scale=0.125)
                pending.append((rt, probs_t))
                if len(pending) > 2:
                    prt, pt = pending.pop(0)
                    emit_ctx(h, prt, pt, cpsAB)
                # bands + evicts + shear for head h+1, tile rt -- emitted
                # last so the previous tile's evicts have freed the bandp
                # bufs by the time the PE reaches these matmuls
                if next_tiles is not None:
                    emit_band_tile(h + 1, rt, next_tiles)
            for prt, pt in pending:
                emit_ctx(h, prt, pt, cpsAB)
            # ship unnormalized ctxT (64 dims + denom row per head) to DRAM
            outsb = work.tile([65, 1024], F32, tag="outsb")
            nc.scalar.copy(outsb[:, 0:512], cpsAB[0][:])
            nc.scalar.copy(outsb[:, 512:1024], cpsAB[1][:])
            nc.sync.dma_start(outd.ap()[65 * h:65 * h + 65, :], outsb[:])

        tiles = head_tiles(0)
        for t in range(NT):
            emit_band_tile(0, t, tiles)
        for h in range(HPC):
            next_tiles = head_tiles(h + 1) if h + 1 < HPC else None
            emit_head(h, tiles, next_tiles)
            tiles = next_tiles


def build_module():
    from concourse import bacc
    nc = bacc.Bacc("TRN2", target_bir_lowering=False)
    io = {
        "hsT": nc.dram_tensor("hsT", [KA, S], BF16, kind="ExternalInput"),
        "wqT": nc.dram_tensor("wqT", [KA, DHC], BF16, kind="ExternalInput"),
        "wkT": nc.dram_tensor("wkT", [KA, DHC], BF16, kind="ExternalInput"),
        "wvT": nc.dram_tensor("wvT", [KA, VW], BF16, kind="ExternalInput"),
        "bq": nc.dram_tensor("bq", [128, 4], F32, kind="ExternalInput"),
        "bk": nc.dram_tensor("bk", [128, 4], F32, kind="ExternalInput"),
        "PT": nc.dram_tensor("PT", [64, 2048], BF16, kind="ExternalInput"),
        "PrT": nc.dram_tensor("PrT", [64, 2048], BF16, kind="ExternalInput"),
        "maskT": nc.dram_tensor("maskT", [128, 8], F32, kind="ExternalInput"),
        "out": nc.dram_tensor("out", [OW, S], F32, kind="ExternalOutput"),
    }
    with tile.TileContext(nc) as tc:
        _emit(tc, io)
    nc.compile()
    return nc


def shard_inputs(hidden_states, attention_mask, wq, bq, wk, bk, wv, bv, dist_emb):
    """Full fp32 inputs -> per-core in_maps (bf16 where appropriate)."""
    hidden_states = np.asarray(hidden_states, np.float32)
    attention_mask = np.asarray(attention_mask, np.float32)
    wq, bq = np.asarray(wq, np.float32), np.asarray(bq, np.float32)
    wk, bk = np.asarray(wk, np.float32), np.asarray(bk, np.float32)
    wv, bv = np.asarray(wv, np.float32), np.asarray(bv, np.float32)
    dist_emb = np.asarray(dist_emb, np.float32)

    PT = np.zeros((64, 2048), bf16)
    PT[:, :2047] = dist_emb.T.astype(bf16)
    PrT = np.zeros((64, 2048), bf16)
    PrT[:, :2047] = dist_emb[::-1].T.astype(bf16)

    in_maps = []
    for c in range(NCORES):
        b, hg = c // 2, c % 2
        sl = slice(DHC * hg, DHC * (hg + 1))

        hsT = np.zeros((KA, S), bf16)
        hsT[:H] = hidden_states[b].T.astype(bf16)
        hsT[H] = bf16(1.0)

        wqT = np.zeros((KA, DHC), bf16)
        wqT[:H] = wq[sl].T.astype(bf16)
        wkT = np.zeros((KA, DHC), bf16)
        wkT[:H] = wk[sl].T.astype(bf16)
        bq_t = np.ascontiguousarray(bq[sl].reshape(4, 128).T).astype(np.float32)
        bk_t = np.ascontiguousarray(bk[sl].reshape(4, 128).T).astype(np.float32)

        wvT = np.zeros((KA, VW), bf16)
        for h in range(HPC):
            cs = 65 * h
            wvT[:H, cs:cs + 64] = wv[DHC * hg + 64 * h:DHC * hg + 64 * h + 64].T.astype(bf16)
            wvT[H, cs:cs + 64] = bv[DHC * hg + 64 * h:DHC * hg + 64 * h + 64].astype(bf16)
            wvT[H, cs + 64] = bf16(1.0)

        # mask rides the EXP activation bias: exp(0.125*sc + mask)
        maskT = np.ascontiguousarray(
            attention_mask[b, 0, 0].reshape(8, 128).T).astype(np.float32)

        in_maps.append({"hsT": hsT, "wqT": wqT, "wkT": wkT, "wvT": wvT,
                        "bq": bq_t, "bk": bk_t,
                        "PT": PT.copy(), "PrT": PrT.copy(), "maskT": maskT})
    return in_maps


def assemble_output(results):
    out = np.zeros((B, S, H), np.float32)
    for c in range(NCORES):
        b, hg = c // 2, c % 2
        r = results[c]["out"]  # [8*65, S] unnormalized ctxT + denom rows
        for h in range(HPC):
            den = r[65 * h + 64]
            out[b, :, DHC * hg + 64 * h:DHC * hg + 64 * h + 64] = \
                (r[65 * h:65 * h + 64] / den).T
    return out


_NC_CACHE = {}


def kernel(**inputs):
    from concourse import bass_utils
    if "nc" not in _NC_CACHE:
        _NC_CACHE["nc"] = build_module()
    nc = _NC_CACHE["nc"]
    in_maps = shard_inputs(**inputs)
    res = bass_utils.run_bass_kernel_spmd(nc, in_maps, core_ids=list(range(NCORES)))
    return assemble_output(res.results)


# revision 64
# speedup vs baseline: 1.0341x; 1.0232x over previous
"""BertSelfAttention (relative_key_query) Trainium2 kernel, 8-core SPMD.

Sharding: core c -> (batch b = c//2, head-group hg = c%2, 8 heads each).
All matmul inputs bf16, PSUM fp32.

Key trick: the relative-position bias terms are Toeplitz.  Per 128-row tile
we compute a "band" matmul U[p, c] = q[l0+p] . P[band + c] on the PE, then
realign it with a single diagonal SBUF->SBUF DMA whose access pattern steps
partition-pitch-minus-one elements per partition (a shear).  The q-side bias
is computed in (l, r) orientation, sheared, then transposed into scores via
PE transpose ops; the k-side bias shears directly into (r, l) orientation.
Scores are built transposed (scoresT[r, l]) so that probs@v needs no
transpose and the softmax denominator falls out of a ones-column in v.

Pipeline layout (this version): single-head software pipeline -- the band
matmuls + evicts + U shears of head h+1 are interleaved into the scores
loop of head h so the PE never drains.  Per rt block: the kq scores land
in two single-bank PSUM halves, the sheared U band is PE-transposed
(bf16) into one psU bank, the V band shears into SBUF ahead of time
(SWDGE, hoisted to head start), and DVE combines them in two adds
(psU+Vsh all-bf16, then psS+sc1 per half -- a TensorTensor may read only
one PSUM operand).  The attention mask rides the EXP activation bias.
Band-piece evicts rotate DVE/ACT/ACT (DVE also owns the adds) and are
emitted after the adds so the adds sit at the DVE queue head.  ctx runs
v-stationary (ctxT[d, l], one matmul per 512 l-cols) with emission
delayed two iterations so the in-order PE queue never waits on EXP; the
output ships unnormalized (64 dims + denominator row per head) and the
softmax division happens on host in assemble_output.

PSUM groups must be balanced (one start, one stop, consumers released at
stop) and a matmul may not cross a PSUM bank boundary; start_tensor_calc
zeroes bank-granularly (2KB).
"""

import numpy as np
import ml_dtypes
from contextlib import ExitStack

import concourse.bass as bass
import concourse.mybir as mybir
import concourse.tile as tile
from concourse.masks import make_identity

bf16 = ml_dtypes.bfloat16
F32 = mybir.dt.float32
BF16 = mybir.dt.bfloat16

B, S, H = 4, 1024, 1024
NH, HD = 16, 64
MAXPOS = 1024
NCORES = 8
HPC = 8          # heads per core
DHC = HPC * HD   # 512 out-dims per core
KA = H + 8       # augmented contraction (bias fold), 1032
NT = S // 128    # 8 tiles of 128 along sequence
BW = 1152        # band width
BPW = 384        # band piece width (3 pieces per band tile, 1 PSUM bank each)
VW = 520         # v block width: 8 heads x 65 (64 dims + ones col)
OW = HPC * 65    # out width per core: 8 heads x (64 dims + denom) = 520


def _emit(tc, io):
    nc = tc.nc
    ctx = ExitStack()
    with ctx:
        # ---------------- persistent tiles ----------------
        pers = ctx.enter_context(tc.tile_pool(name="pers", bufs=1))
        # P tables replicated into both 64-partition halves so that
        # band/qk matmuls can match any head's base partition.
        PT_sb = pers.tile([128, 2048], BF16)
        PrT_sb = pers.tile([128, 2048], BF16)
        ident = pers.tile([128, 128], BF16)
        mask_sb = pers.tile([128, 8], F32)
        qT_sb = pers.tile([128, 4 * 1024], BF16)   # dh-chunk c at cols 1024c
        kT_sb = pers.tile([128, 4 * 1024], BF16)
        v_sb = pers.tile([128, NT * VW], BF16)     # r-chunk rc at cols VW*rc

        nc.sync.dma_start(PT_sb[0:64, :], io["PT"][:])
        nc.sync.dma_start(PT_sb[64:128, :], io["PT"][:])
        nc.sync.dma_start(PrT_sb[0:64, :], io["PrT"][:])
        nc.sync.dma_start(PrT_sb[64:128, :], io["PrT"][:])
        nc.sync.dma_start(mask_sb[:], io["maskT"][:])
        make_identity(nc, ident[:])

        # ---------------- projections ----------------
        with tc.tile_pool(name="proj", bufs=1) as pj, \
             tc.tile_pool(name="projps", bufs=2, space="PSUM") as pjps:
            hs_m = pj.tile([128, 8 * 1024], BF16)   # k-chunk kc at cols 1024kc
            hs_t = pj.tile([8, 1024], BF16)
            wq_m = pj.tile([128, 8 * DHC], BF16)
            wk_m = pj.tile([128, 8 * DHC], BF16)
            wv_m = pj.tile([128, 8 * VW], BF16)
            wv_t = pj.tile([8, VW], BF16)
            bq_sb = pj.tile([128, 4], F32)
            bk_sb = pj.tile([128, 4], F32)

            hsd, wqd, wkd, wvd = io["hsT"], io["wqT"], io["wkT"], io["wvT"]
            # hsT dram [1032, 1024] -> chunked SBUF layout
            nc.sync.dma_start(
                hs_m[:],
                bass.AP(hsd.ap().tensor, 0, [[1024, 128], [128 * 1024, 8], [1, 1024]]))
            nc.sync.dma_start(hs_t[:], hsd.ap()[1024:1032, :])
            for wm, wd, wcols in ((wq_m, wqd, DHC), (wk_m, wkd, DHC),
                                  (wv_m, wvd, VW)):
                nc.sync.dma_start(
                    wm[:],
                    bass.AP(wd.ap().tensor, 0,
                            [[wcols, 128], [128 * wcols, 8], [1, wcols]]))
            nc.sync.dma_start(wv_t[:], wvd.ap()[1024:1032, :])
            nc.sync.dma_start(bq_sb[:], io["bq"].ap()[:])
            nc.sync.dma_start(bk_sb[:], io["bk"].ap()[:])

            # q/k: out[dh-chunk c partitions, tokens]; bias rides the evict
            # activation (per-partition = per-out-dim)
            for wm, bias, dst in ((wq_m, bq_sb, qT_sb), (wk_m, bk_sb, kT_sb)):
                for c in range(4):
                    for th in range(2):
                        ps = pjps.tile([128, 512], F32, tag="pps")
                        for kc in range(8):
                            nc.tensor.matmul(
                                ps[:],
                                wm[:, 512 * kc + 128 * c:512 * kc + 128 * c + 128],
                                hs_m[:, 1024 * kc + 512 * th:1024 * kc + 512 * th + 512],
                                start=(kc == 0), stop=(kc == 7))
                        nc.scalar.activation(
                            dst[:, 1024 * c + 512 * th:1024 * c + 512 * th + 512],
                            ps[:], mybir.ActivationFunctionType.Identity,
                            bias=bias[:, c:c + 1])
            # v: out[token-chunk rc partitions, VW]
            for rc in range(8):
                psa = pjps.tile([128, 512], F32, tag="pps")
                psb = pjps.tile([128, 8], F32, tag="ppsb")
                for kc in range(8):
                    lhs = hs_m[:, 1024 * kc + 128 * rc:1024 * kc + 128 * rc + 128]
                    nc.tensor.matmul(psa[:], lhs,
                                     wv_m[:, VW * kc:VW * kc + 512],
                                     start=(kc == 0), stop=False)
                    nc.tensor.matmul(psb[:], lhs,
                                     wv_m[:, VW * kc + 512:VW * kc + VW],
                                     start=(kc == 0), stop=False)
                nc.tensor.matmul(psa[:], hs_t[:, 128 * rc:128 * rc + 128],
                                 wv_t[:, 0:512], start=False, stop=True)
                nc.tensor.matmul(psb[:], hs_t[:, 128 * rc:128 * rc + 128],
                                 wv_t[:, 512:VW], start=False, stop=True)
                nc.vector.tensor_copy(v_sb[:, VW * rc:VW * rc + 512], psa[:])
                nc.vector.tensor_copy(v_sb[:, VW * rc + 512:VW * rc + VW], psb[:])

        # ---------------- per-head attention ----------------
        # PSUM budget (8 banks): band pieces f32[128,384] x3bufs = 3,
        # psS halves f32[128,512] x2 = 2, psU bf16[128,1024] x1 = 1,
        # ctx f32[65,512] x2 = 2.
        bands = ctx.enter_context(tc.tile_pool(name="bands", bufs=2))
        ubp = ctx.enter_context(tc.tile_pool(name="ubp", bufs=3))
        shrd = ctx.enter_context(tc.tile_pool(name="shrd", bufs=2))
        work = ctx.enter_context(tc.tile_pool(name="work", bufs=2))
        bandp = ctx.enter_context(tc.tile_pool(name="bandp", bufs=3, space="PSUM"))
        sps = ctx.enter_context(tc.tile_pool(name="sps", bufs=2, space="PSUM"))
        ups = ctx.enter_context(tc.tile_pool(name="ups", bufs=1, space="PSUM"))
        ctxps = ctx.enter_context(tc.tile_pool(name="ctxps", bufs=1, space="PSUM"))

        outd = io["out"]
        # GPSIMD cannot access PSUM (walrus verifier) -> evicts ride DVE/ACT.
        # DVE also carries the bias adds, so it only gets 1 piece in 3.
        evict_engines = (nc.scalar, nc.vector, nc.scalar)
        evict_n = [0]

        def head_tiles(h):
            Vb = bands.tile([128, NT * BW], BF16, tag="Vb")
            Ush = shrd.tile([128, NT * 1024], BF16, tag="Ush")
            Vsh = shrd.tile([128, NT * 1024], BF16, tag="Vsh")
            return (Vb, Ush, Vsh)

        def emit_band_tile(h, t, tiles):
            """Band matmuls + evicts for tile t of head h, then the U shear."""
            Vb, Ush, Vsh = tiles
            hc, ho = h // 2, 64 * (h % 2)
            s0 = 896 - 128 * t
            Ubt = ubp.tile([128, BW], BF16, tag="Ubt")
            for src_sb, tbl, dst in ((qT_sb, PrT_sb, None), (kT_sb, PT_sb, Vb)):
                lhsT = src_sb[ho:ho + 64, 1024 * hc + 128 * t:1024 * hc + 128 * t + 128]
                for k in range(3):
                    ps = bandp.tile([128, BPW], F32, tag="bp")
                    nc.tensor.matmul(ps[:], lhsT,
                                     tbl[ho:ho + 64, s0 + BPW * k:s0 + BPW * (k + 1)])
                    eng = evict_engines[evict_n[0] % 3]
                    evict_n[0] += 1
                    if dst is None:
                        dpiece = Ubt[:, BPW * k:BPW * (k + 1)]
                    else:
                        dpiece = dst[:, BW * t + BPW * k:BW * t + BPW * (k + 1)]
                    if eng is nc.scalar:
                        eng.copy(dpiece, ps[:])
                    else:
                        eng.tensor_copy(dpiece, ps[:])
                if dst is None:
                    # U shear tile t: partition p reads cols (127 - p ..)
                    sap = Ubt[:]
                    diag = bass.AP(sap.tensor, sap.offset + 127,
                                   [[BW - 1, 128], [1, 1024]])
                    dap = Ush[:]
                    dstap = bass.AP(dap.tensor, dap.offset + 1024 * t,
                                    [[NT * 1024, 128], [1, 1024]])
                    nc.sync.dma_start(dstap, diag)

        def emit_ctx(h, rt, probs_t, cpsAB):
            # v-stationary orientation: ctxT[d, l] = sum_r v[r, d] probsT[r, l]
            # accumulated over rt blocks.  One matmul per 512 l-columns (one
            # PSUM bank each), balanced start/stop per tile.
            for hf in range(2):
                nc.tensor.matmul(
                    cpsAB[hf][:],
                    v_sb[:, VW * rt + 65 * h:VW * rt + 65 * h + 65],
                    probs_t[:, 512 * hf:512 * hf + 512],
                    start=(rt == 0), stop=(rt == NT - 1))

        def emit_head(h, tiles, next_tiles):
            """Scores/softmax/ctx for head h; bands for head h+1 interleaved."""
            Vb, Ush, Vsh = tiles
            hc, ho = h // 2, 64 * (h % 2)
            cpsA = ctxps.tile([65, 512], F32, tag="ctxA")
            cpsB = ctxps.tile([65, 512], F32, tag="ctxB")
            cpsAB = (cpsA, cpsB)
            # V bias shears: all inputs (Vb) were evicted during head h-1,
            # so hoist the SWDGE work ahead of the rt loop
            for rt in range(NT):
                sap = Vb[:]
                diag = bass.AP(sap.tensor, sap.offset + BW * rt + 127,
                               [[NT * BW - 1, 128], [1, 1024]])
                nc.gpsimd.dma_start(Vsh[:, 1024 * rt:1024 * rt + 1024], diag)
            pending = []  # (rt, probs_tile) awaiting ctx emission (depth 2)
            for rt in range(NT):
                # scoresT (k . q) per 512-half
                lhsT = kT_sb[ho:ho + 64, 1024 * hc + 128 * rt:1024 * hc + 128 * rt + 128]
                halves = []
                for hf in range(2):
                    psS = sps.tile([128, 512], F32, tag="S")
                    nc.tensor.matmul(
                        psS[:], lhsT,
                        qT_sb[ho:ho + 64, 1024 * hc + 512 * hf:1024 * hc + 512 * hf + 512])
                    halves.append(psS)
                # transposed U bias for this rt: bf16 PE transposes into one
                # psU bank (first carries start, bank zeroing covers all)
                psU = ups.tile([128, 1024], BF16, tag="U")
                for lt in range(NT):
                    nc.tensor.matmul(
                        psU[:, 128 * lt:128 * lt + 128],
                        Ush[:, 1024 * lt + 128 * rt:1024 * lt + 128 * rt + 128],
                        ident[:], is_transpose=True,
                        start=(lt == 0), stop=(lt == NT - 1))
                # sc1 = psU + Vsh (all-bf16 DVE add), sc = psS + sc1, exp.
                # Emitted BEFORE the band evicts so the adds sit at the DVE
                # queue head and release psS/psU promptly.
                sc1 = work.tile([128, 1024], BF16, tag="sc1")
                nc.vector.tensor_add(sc1[:], psU[:],
                                     Vsh[:, 1024 * rt:1024 * rt + 1024])
                sc = work.tile([128, 1024], BF16, tag="sc")
                for hf in range(2):
                    nc.vector.tensor_add(sc[:, 512 * hf:512 * hf + 512],
                                         halves[hf][:],
                                         sc1[:, 512 * hf:512 * hf + 512])
                probs_t = work.tile([128, 1024], BF16, tag="probs", bufs=4)
                nc.scalar.activation(probs_t[:], sc[:],
                                     mybir.ActivationFunctionType.Exp,
                                     bias=mask_sb[:, rt:rt + 1], scale=0.125)
                pending.append((rt, probs_t))
                if len(pending) > 2:
                    prt, pt = pending.pop(0)
                    emit_ctx(h, prt, pt, cpsAB)
                # bands + evicts + shear for head h+1, tile rt -- emitted
                # last so the previous tile's evicts have freed the bandp
                # bufs by the time the PE reaches these matmuls
                if next_tiles is not None:
                    emit_band_tile(h + 1, rt, next_tiles)
            for prt, pt in pending:
                emit_ctx(h, prt, pt, cpsAB)
            # ship unnormalized ctxT (64 dims + denom row per head) to DRAM
            outsb = work.tile([65, 1024], F32, tag="outsb")
            nc.scalar.copy(outsb[:, 0:512], cpsAB[0][:])
            nc.scalar.copy(outsb[:, 512:1024], cpsAB[1][:])
            nc.sync.dma_start(outd.ap()[65 * h:65 * h + 65, :], outsb[:])

        tiles = head_tiles(0)
        for t in range(NT):
            emit_band_tile(0, t, tiles)
        for h in range(HPC):
            next_tiles = head_tiles(h + 1) if h + 1 < HPC else None
            emit_head(h, tiles, next_tiles)
            tiles = next_tiles


def build_module():
    from concourse import bacc
    nc = bacc.Bacc("TRN2", target_bir_lowering=False)
    io = {
        "hsT": nc.dram_tensor("hsT", [KA, S], BF16, kind="ExternalInput"),
        "wqT": nc.dram_tensor("wqT", [KA, DHC], BF16, kind="ExternalInput"),
        "wkT": nc.dram_tensor("wkT", [KA, DHC], BF16, kind="ExternalInput"),
        "wvT": nc.dram_tensor("wvT", [KA, VW], BF16, kind="ExternalInput"),
        "bq": nc.dram_tensor("bq", [128, 4], F32, kind="ExternalInput"),
        "bk": nc.dram_tensor("bk", [128, 4], F32, kind="ExternalInput"),
        "PT": nc.dram_tensor("PT", [64, 2048], BF16, kind="ExternalInput"),
        "PrT": nc.dram_tensor("PrT", [64, 2048], BF16, kind="ExternalInput"),
        "maskT": nc.dram_tensor("maskT", [128, 8], F32, kind="ExternalInput"),
        "out": nc.dram_tensor("out", [OW, S], F32, kind="ExternalOutput"),
    }
    with tile.TileContext(nc) as tc:
        _emit(tc, io)
    nc.compile()
    return nc


def shard_inputs(hidden_states, attention_mask, wq, bq, wk, bk, wv, bv, dist_emb):
    """Full fp32 inputs -> per-core in_maps (bf16 where appropriate)."""
    hidden_states = np.asarray(hidden_states, np.float32)
    attention_mask = np.asarray(attention_mask, np.float32)
    wq, bq = np.asarray(wq, np.float32), np.asarray(bq, np.float32)
    wk, bk = np.asarray(wk, np.float32), np.asarray(bk, np.float32)
    wv, bv = np.asarray(wv, np.float32), np.asarray(bv, np.float32)
    dist_emb = np.asarray(dist_emb, np.float32)

    PT = np.zeros((64, 2048), bf16)
    PT[:, :2047] = dist_emb.T.astype(bf16)
    PrT = np.zeros((64, 2048), bf16)
    PrT[:, :2047] = dist_emb[::-1].T.astype(bf16)

    in_maps = []
    for c in range(NCORES):
        b, hg = c // 2, c % 2
        sl = slice(DHC * hg, DHC * (hg + 1))

        hsT = np.zeros((KA, S), bf16)
        hsT[:H] = hidden_states[b].T.astype(bf16)
        hsT[H] = bf16(1.0)

        wqT = np.zeros((KA, DHC), bf16)
        wqT[:H] = wq[sl].T.astype(bf16)
        wkT = np.zeros((KA, DHC), bf16)
        wkT[:H] = wk[sl].T.astype(bf16)
        bq_t = np.ascontiguousarray(bq[sl].reshape(4, 128).T).astype(np.float32)
        bk_t = np.ascontiguousarray(bk[sl].reshape(4, 128).T).astype(np.float32)

        wvT = np.zeros((KA, VW), bf16)
        for h in range(HPC):
            cs = 65 * h
            wvT[:H, cs:cs + 64] = wv[DHC * hg + 64 * h:DHC * hg + 64 * h + 64].T.astype(bf16)
            wvT[H, cs:cs + 64] = bv[DHC * hg + 64 * h:DHC * hg + 64 * h + 64].astype(bf16)
            wvT[H, cs + 64] = bf16(1.0)

        # mask rides the EXP activation bias: exp(0.125*sc + mask)
        maskT = np.ascontiguousarray(
            attention_mask[b, 0, 0].reshape(8, 128).T).astype(np.float32)

        in_maps.append({"hsT": hsT, "wqT": wqT, "wkT": wkT, "wvT": wvT,
                        "bq": bq_t, "bk": bk_t,
                        "PT": PT.copy(), "PrT": PrT.copy(), "maskT": maskT})
    return in_maps


def assemble_output(results):
    out = np.zeros((B, S, H), np.float32)
    for c in range(NCORES):
        b, hg = c // 2, c % 2
        r = results[c]["out"]  # [8*65, S] unnormalized ctxT + denom rows
        for h in range(HPC):
            den = r[65 * h + 64]
            out[b, :, DHC * hg + 64 * h:DHC * hg + 64 * h + 64] = \
                (r[65 * h:65 * h + 64] / den).T
    return out


_NC_CACHE = {}


def kernel(**inputs):
    from concourse import bass_utils
    if "nc" not in _NC_CACHE:
        _NC_CACHE["nc"] = build_module()
    nc = _NC_CACHE["nc"]
    in_maps = shard_inputs(**inputs)
    res = bass_utils.run_bass_kernel_spmd(nc, in_maps, core_ids=list(range(NCORES)))
    return assemble_output(res.results)
